# revision 4
# baseline (speedup 1.0000x reference)
"""GraphVAE (2x GCNConv + inner-product decoder) on 8 TRN2 NeuronCores.

Strategy:
  - Host: build dense normalized adjacency ST[j,i] = sum_{e:(j->i)} dis[j]*dis[i]
    (includes the doubled self-loops), cast to bf16, shard columns (= output
    nodes) across the 8 cores. Transpose x to feat-major bf16.
  - Device (SPMD, per core c owning nodes I_c = [1024c, 1024(c+1))):
      h    = x @ W1                      (replicated, node-major bf16)
      z1T  = relu(sum_j h[j] ST_c[j] + b1)   (feat-major, own cols)
      zw_c = z1_c @ W2                   -> AllGather -> zw (node-major)
      zT   = sum_j zw[j] ST_c[j] + b2    (feat-major)
      muT  = Wmu.T zT + bmu ; lvT = Wlv.T zT + blv
      mu_c / lv_c via PE transpose (fp32 outputs)
      muT bf16 -> AllGather -> muT_all
      adj_c = sigmoid(muT_c.T @ muT_all)  (1024 x 8192, fp32 out)
All matmuls bf16 with fp32 PSUM accumulation.
"""

import os
import sys
import types

import ml_dtypes
import numpy as np

N = 8192
D_IN = 512
H1 = 256
LAT = 128
NC = 8
NB = N // 128          # 64 node blocks of 128
OWN = N // NC          # 1024 nodes per core
OWNB = OWN // 128      # 8 own node blocks

BF16 = ml_dtypes.bfloat16

_CACHE = {}
LAST_RESULT = None


def _install_ntff_shim():
    # Allows BASS_TRACE=1 profiling under axon (optional; harmless otherwise).
    try:
        import antenv.axon_hooks  # noqa: F401
        return
    except ImportError:
        pass
    try:
        from trn_agent_boot.trn_boot import _ntff_profile_via_ctypes
        hook = _ntff_profile_via_ctypes("/opt/axon/libaxon_pjrt.so")
        mod = types.ModuleType("antenv.axon_hooks")
        mod.get_axon_ntff_profile_hook = lambda: hook
        sys.modules["antenv.axon_hooks"] = mod
    except Exception:
        pass


def _build_module():
    import concourse.bass as bass
    import concourse.tile as tile
    from concourse import bacc, mybir
    from concourse.masks import make_identity

    f32 = mybir.dt.float32
    bf = mybir.dt.bfloat16
    AF = mybir.ActivationFunctionType

    nc = bacc.Bacc("TRN2", target_bir_lowering=False, debug=False, num_devices=NC)

    xT_in = nc.dram_tensor("xT", [D_IN, N], bf, kind="ExternalInput")
    st_in = nc.dram_tensor("STc", [N, OWN], bf, kind="ExternalInput")
    w1_in = nc.dram_tensor("W1", [D_IN, H1], bf, kind="ExternalInput")
    w2_in = nc.dram_tensor("W2", [H1, LAT], bf, kind="ExternalInput")
    wmu_in = nc.dram_tensor("Wmu", [LAT, LAT], bf, kind="ExternalInput")
    wlv_in = nc.dram_tensor("Wlv", [LAT, LAT], bf, kind="ExternalInput")
    b1_in = nc.dram_tensor("b1", [H1, 1], f32, kind="ExternalInput")
    b2_in = nc.dram_tensor("b2", [LAT, 1], f32, kind="ExternalInput")
    bmu_in = nc.dram_tensor("bmu", [LAT, 1], f32, kind="ExternalInput")
    blv_in = nc.dram_tensor("blv", [LAT, 1], f32, kind="ExternalInput")

    adj_out = nc.dram_tensor("adj", [OWN, N], f32, kind="ExternalOutput")
    mu_out = nc.dram_tensor("mu", [OWN, LAT], f32, kind="ExternalOutput")
    lv_out = nc.dram_tensor("lv", [OWN, LAT], f32, kind="ExternalOutput")

    rg = [list(range(NC))]

    with tile.TileContext(nc) as tc:
        from contextlib import ExitStack
        ctx = ExitStack()
        with ctx:
            pers = ctx.enter_context(tc.tile_pool(name="pers", bufs=1))
            dram = ctx.enter_context(tc.tile_pool(name="dram", bufs=1, space="DRAM"))

            # --- weights to SBUF ---
            w1b = pers.tile([128, 4, H1], bf)       # [d-part, dblk, f1]
            nc.sync.dma_start(w1b[:], w1_in.ap().rearrange("(db p) f -> p db f", p=128))
            w2b = pers.tile([128, 2, LAT], bf)      # [f-part, fblk, n]
            nc.sync.dma_start(w2b[:], w2_in.ap().rearrange("(fb p) n -> p fb n", p=128))
            wmub = pers.tile([128, LAT], bf)
            nc.sync.dma_start(wmub[:], wmu_in.ap())
            wlvb = pers.tile([128, LAT], bf)
            nc.sync.dma_start(wlvb[:], wlv_in.ap())
            b1sb = pers.tile([128, 2], f32)         # [f-part, fblk]
            nc.sync.dma_start(b1sb[:], b1_in.ap().rearrange("(fb p) o -> p fb o", p=128))
            b2sb = pers.tile([128, 1], f32)
            nc.sync.dma_start(b2sb[:], b2_in.ap())
            bmusb = pers.tile([128, 1], f32)
            nc.sync.dma_start(bmusb[:], bmu_in.ap())
            blvsb = pers.tile([128, 1], f32)
            nc.sync.dma_start(blvsb[:], blv_in.ap())
            ident = pers.tile([128, 128], f32)
            make_identity(nc, ident[:])

            z1t = pers.tile([128, 2, OWN], bf)      # [f-part, fblk, own-i]
            zwall = pers.tile([128, NB, LAT], bf)   # [j-part, jb, n]
            ztb = pers.tile([128, OWN], bf)         # [f2-part, own-i]
            zwb = pers.tile([128, OWNB, LAT], bf)   # own zw, [j-part, jb, n]

            zw_bounce = dram.tile([OWN, LAT], bf)
            zw_all_d = dram.tile([N, LAT], bf, addr_space="Shared")
            mu_bounce = dram.tile([128, OWN], bf)
            mu_all_d = dram.tile([NC * 128, OWN], bf, addr_space="Shared")

            with tc.tile_pool(name="big", bufs=1) as big:
                stF = big.tile([128, NB, OWN], bf)  # [j-part, jb, own-i] 128K/part
                hb = big.tile([128, NB, H1], bf)    # [j-part, jb, f1]    32K/part

                # ST load in 4 chunks of 16 jb (4 MB each) so conv1 can start early
                for q in range(4):
                    nc.sync.dma_start(
                        stF[:, q * 16:(q + 1) * 16, :],
                        st_in.ap()[q * 2048:(q + 1) * 2048, :]
                        .rearrange("(jb p) i -> p jb i", p=128),
                    )

                # --- phase A: h = x @ W1 (node-major, all 64 blocks) ---
                with (
                    tc.tile_pool(name="xt", bufs=2) as xpool,
                    tc.tile_pool(name="pa", bufs=4, space="PSUM") as pa,
                ):
                    for g in range(16):  # groups of 4 node blocks
                        xt = xpool.tile([128, 4, 512], bf)
                        nc.sync.dma_start(
                            xt[:],
                            xT_in.ap()[:, g * 512:(g + 1) * 512]
                            .rearrange("(db p) j -> p db j", p=128),
                        )
                        for jj in range(4):
                            jb = g * 4 + jj
                            ph = pa.tile([128, H1], f32)
                            for db in range(4):
                                nc.tensor.matmul(
                                    ph[:],
                                    xt[:, db, jj * 128:(jj + 1) * 128],
                                    w1b[:, db, :],
                                    start=(db == 0),
                                    stop=(db == 3),
                                )
                            nc.scalar.copy(hb[:, jb, :], ph[:])

                # --- phase B: conv1 -> z1T = relu(sum_j h[j]^T . ST[j] + b1) ---
                with tc.tile_pool(name="pb", bufs=4, space="PSUM") as pb:
                    for fblk in range(2):
                        for ch in range(2):
                            pz = pb.tile([128, 512], f32)
                            for jb in range(NB):
                                nc.tensor.matmul(
                                    pz[:],
                                    hb[:, jb, fblk * 128:(fblk + 1) * 128],
                                    stF[:, jb, ch * 512:(ch + 1) * 512],
                                    start=(jb == 0),
                                    stop=(jb == NB - 1),
                                )
                            nc.scalar.activation(
                                z1t[:, fblk, ch * 512:(ch + 1) * 512],
                                pz[:],
                                AF.Relu,
                                bias=b1sb[:, fblk:fblk + 1],
                            )

                # --- phase C: zw_c = z1_c @ W2 ; AllGather ---
                with tc.tile_pool(name="pc", bufs=2, space="PSUM") as pc:
                    for jj in range(OWNB):
                        pw = pc.tile([128, LAT], f32)
                        for fblk in range(2):
                            nc.tensor.matmul(
                                pw[:],
                                z1t[:, fblk, jj * 128:(jj + 1) * 128],
                                w2b[:, fblk, :],
                                start=(fblk == 0),
                                stop=(fblk == 1),
                            )
                        nc.scalar.copy(zwb[:, jj, :], pw[:])
                nc.sync.dma_start(
                    zw_bounce[:].rearrange("(jb p) n -> p jb n", p=128), zwb[:]
                )
                nc.gpsimd.collective_compute(
                    "AllGather",
                    bass.mybir.AluOpType.bypass,
                    replica_groups=rg,
                    ins=[zw_bounce.opt()],
                    outs=[zw_all_d.opt()],
                )
                nc.sync.dma_start(
                    zwall[:], zw_all_d[:].rearrange("(jb p) n -> p jb n", p=128)
                )

                # --- phase D: conv2 -> zT = sum_j zw[j]^T . ST[j] + b2 ---
                with tc.tile_pool(name="pd", bufs=2, space="PSUM") as pd:
                    for ch in range(2):
                        pz2 = pd.tile([128, 512], f32)
                        for jb in range(NB):
                            nc.tensor.matmul(
                                pz2[:],
                                zwall[:, jb, :],
                                stF[:, jb, ch * 512:(ch + 1) * 512],
                                start=(jb == 0),
                                stop=(jb == NB - 1),
                            )
                        nc.scalar.activation(
                            ztb[:, ch * 512:(ch + 1) * 512],
                            pz2[:],
                            AF.Identity,
                            bias=b2sb[:],
                        )

            # big pool (ST, hb) released here
            with tc.tile_pool(name="post", bufs=1) as post:
                muT = post.tile([128, OWN], f32)
                lvT = post.tile([128, OWN], f32)
                muTb = post.tile([128, OWN], bf)
                musb = post.tile([128, OWNB, LAT], f32)
                lvsb = post.tile([128, OWNB, LAT], f32)
                muall = post.tile([128, NC, OWN], bf)

                # --- phase E: muT/lvT + transposes ---
                with tc.tile_pool(name="pe", bufs=4, space="PSUM") as pe:
                    for ch in range(2):
                        pm = pe.tile([128, 512], f32)
                        nc.tensor.matmul(
                            pm[:], wmub[:], ztb[:, ch * 512:(ch + 1) * 512],
                            start=True, stop=True,
                        )
                        nc.scalar.activation(
                            muT[:, ch * 512:(ch + 1) * 512], pm[:],
                            AF.Identity, bias=bmusb[:],
                        )
                        pl = pe.tile([128, 512], f32)
                        nc.tensor.matmul(
                            pl[:], wlvb[:], ztb[:, ch * 512:(ch + 1) * 512],
                            start=True, stop=True,
                        )
                        nc.scalar.activation(
                            lvT[:, ch * 512:(ch + 1) * 512], pl[:],
                            AF.Identity, bias=blvsb[:],
                        )
                    nc.vector.tensor_copy(muTb[:], muT[:])

                with tc.tile_pool(name="pt", bufs=4, space="PSUM") as pt:
                    for jj in range(OWNB):
                        ptm = pt.tile([128, 128], f32)
                        nc.tensor.transpose(
                            ptm[:], muT[:, jj * 128:(jj + 1) * 128], ident[:]
                        )
                        nc.scalar.copy(musb[:, jj, :], ptm[:])
                        ptl = pt.tile([128, 128], f32)
                        nc.tensor.transpose(
                            ptl[:], lvT[:, jj * 128:(jj + 1) * 128], ident[:]
                        )
                        nc.scalar.copy(lvsb[:, jj, :], ptl[:])
                nc.sync.dma_start(
                    mu_out.ap().rearrange("(jb p) n -> p jb n", p=128), musb[:]
                )
                nc.sync.dma_start(
                    lv_out.ap().rearrange("(jb p) n -> p jb n", p=128), lvsb[:]
                )

                nc.sync.dma_start(mu_bounce[:], muTb[:])
                nc.gpsimd.collective_compute(
                    "AllGather",
                    bass.mybir.AluOpType.bypass,
                    replica_groups=rg,
                    ins=[mu_bounce.opt()],
                    outs=[mu_all_d.opt()],
                )
                nc.sync.dma_start(
                    muall[:], mu_all_d[:].rearrange("(r p) i -> p r i", p=128)
                )

                # --- phase F: adj_c = sigmoid(muT_c^T . muT_all) ---
                with (
                    tc.tile_pool(name="adjp", bufs=3) as adjp,
                    tc.tile_pool(name="pf", bufs=4, space="PSUM") as pf,
                ):
                    for iblk in range(OWNB):
                        for cg in range(4):  # column groups of 2048
                            adjch = adjp.tile([128, 2048], f32)
                            for s in range(4):
                                col = cg * 2048 + s * 512
                                r8, ch2 = col // 1024, (col % 1024) // 512
                                pa2 = pf.tile([128, 512], f32)
                                nc.tensor.matmul(
                                    pa2[:],
                                    muTb[:, iblk * 128:(iblk + 1) * 128],
                                    muall[:, r8, ch2 * 512:(ch2 + 1) * 512],
                                    start=True, stop=True,
                                )
                                nc.scalar.activation(
                                    adjch[:, s * 512:(s + 1) * 512], pa2[:],
                                    AF.Sigmoid,
                                )
                            nc.sync.dma_start(
                                adj_out.ap()[iblk * 128:(iblk + 1) * 128,
                                             cg * 2048:(cg + 1) * 2048],
                                adjch[:],
                            )

    nc.compile()
    return nc


def _prep_host(x, edge_index, W1, b1, W2, b2, Wmu, bmu, Wlv, blv):
    x = np.asarray(x)
    e = np.asarray(edge_index)
    sl = np.arange(N, dtype=e.dtype)
    r = np.concatenate([e[0], sl, sl])
    c = np.concatenate([e[1], sl, sl])
    deg = np.bincount(c, minlength=N).astype(np.float32)
    dis = np.where(deg > 0, deg ** -0.5, 0.0).astype(np.float32)
    vals = dis[r] * dis[c]
    ST = np.zeros((N, N), dtype=np.float32)
    np.add.at(ST, (r, c), vals)
    STb = ST.astype(BF16)
    xT = np.ascontiguousarray(np.asarray(x, np.float32).T).astype(BF16)

    common = {
        "xT": xT,
        "W1": np.asarray(W1, np.float32).astype(BF16),
        "W2": np.asarray(W2, np.float32).astype(BF16),
        "Wmu": np.asarray(Wmu, np.float32).astype(BF16),
        "Wlv": np.asarray(Wlv, np.float32).astype(BF16),
        "b1": np.asarray(b1, np.float32).reshape(H1, 1),
        "b2": np.asarray(b2, np.float32).reshape(LAT, 1),
        "bmu": np.asarray(bmu, np.float32).reshape(LAT, 1),
        "blv": np.asarray(blv, np.float32).reshape(LAT, 1),
    }
    in_maps = []
    for cidx in range(NC):
        m = dict(common)
        m["STc"] = np.ascontiguousarray(STb[:, cidx * OWN:(cidx + 1) * OWN])
        in_maps.append(m)
    return in_maps


def kernel(x, edge_index, W1, b1, W2, b2, Wmu, bmu, Wlv, blv):
    global LAST_RESULT
    _install_ntff_shim()
    from concourse import bass_utils

    if "nc" not in _CACHE:
        _CACHE["nc"] = _build_module()
    nc = _CACHE["nc"]

    in_maps = _prep_host(x, edge_index, W1, b1, W2, b2, Wmu, bmu, Wlv, blv)
    res = bass_utils.run_bass_kernel_spmd(nc, in_maps, core_ids=list(range(NC)))
    LAST_RESULT = res

    adj = np.concatenate([res.results[c]["adj"] for c in range(NC)], axis=0)
    mu = np.concatenate([res.results[c]["mu"] for c in range(NC)], axis=0)
    lv = np.concatenate([res.results[c]["lv"] for c in range(NC)], axis=0)
    return adj, mu, lv


# revision 10
# speedup vs baseline: 1.1043x; 1.1043x over previous
"""GraphVAE (2x GCNConv + inner-product decoder) on 8 TRN2 NeuronCores.

Strategy:
  - Host: build dense normalized adjacency ST[j,i] = sum_{e:(j->i)} dis[j]*dis[i]
    (includes the doubled self-loops), cast to bf16, shard columns (= output
    nodes) across the 8 cores. Transpose x to feat-major bf16.
  - Device (SPMD, per core c owning nodes I_c = [1024c, 1024(c+1))):
      h    = x @ W1                      (replicated, node-major bf16)
      z1T  = relu(sum_j h[j] ST_c[j] + b1)   (feat-major, own cols)
      zw_c = z1_c @ W2                   -> AllGather -> zw (node-major)
      zT   = sum_j zw[j] ST_c[j] + b2    (feat-major)
      muT  = Wmu.T zT + bmu ; lvT = Wlv.T zT + blv
      mu_c / lv_c via PE transpose (fp32 outputs)
      muT bf16 -> AllGather -> muT_all
      adj_c = sigmoid(muT_c.T @ muT_all)  (1024 x 8192, fp32 out)
All matmuls bf16 with fp32 PSUM accumulation.
"""

import os
import sys
import types

import ml_dtypes
import numpy as np

N = 8192
D_IN = 512
H1 = 256
LAT = 128
NC = 8
NB = N // 128          # 64 node blocks of 128
OWN = N // NC          # 1024 nodes per core
OWNB = OWN // 128      # 8 own node blocks

BF16 = ml_dtypes.bfloat16

_CACHE = {}
LAST_RESULT = None


def _install_ntff_shim():
    # Allows BASS_TRACE=1 profiling under axon (optional; harmless otherwise).
    try:
        import antenv.axon_hooks  # noqa: F401
        return
    except ImportError:
        pass
    try:
        from trn_agent_boot.trn_boot import _ntff_profile_via_ctypes
        hook = _ntff_profile_via_ctypes("/opt/axon/libaxon_pjrt.so")
        mod = types.ModuleType("antenv.axon_hooks")
        mod.get_axon_ntff_profile_hook = lambda: hook
        sys.modules["antenv.axon_hooks"] = mod
    except Exception:
        pass


def _build_module():
    import concourse.bass as bass
    import concourse.tile as tile
    from concourse import bacc, mybir
    from concourse.masks import make_identity

    f32 = mybir.dt.float32
    bf = mybir.dt.bfloat16
    AF = mybir.ActivationFunctionType

    nc = bacc.Bacc("TRN2", target_bir_lowering=False, debug=False, num_devices=NC)

    xT_in = nc.dram_tensor("xT", [D_IN, N], bf, kind="ExternalInput")
    st_in = nc.dram_tensor("STc", [N, OWN], bf, kind="ExternalInput")
    w1_in = nc.dram_tensor("W1", [D_IN, H1], bf, kind="ExternalInput")
    w2_in = nc.dram_tensor("W2", [H1, LAT], bf, kind="ExternalInput")
    wmu_in = nc.dram_tensor("Wmu", [LAT, LAT], bf, kind="ExternalInput")
    wlv_in = nc.dram_tensor("Wlv", [LAT, LAT], bf, kind="ExternalInput")
    b1_in = nc.dram_tensor("b1", [H1, 1], f32, kind="ExternalInput")
    b2_in = nc.dram_tensor("b2", [LAT, 1], f32, kind="ExternalInput")
    bmu_in = nc.dram_tensor("bmu", [LAT, 1], f32, kind="ExternalInput")
    blv_in = nc.dram_tensor("blv", [LAT, 1], f32, kind="ExternalInput")

    adj_out = nc.dram_tensor("adj", [OWN, N], f32, kind="ExternalOutput")
    mu_out = nc.dram_tensor("mu", [OWN, LAT], f32, kind="ExternalOutput")
    lv_out = nc.dram_tensor("lv", [OWN, LAT], f32, kind="ExternalOutput")

    rg = [list(range(NC))]

    with tile.TileContext(nc) as tc:
        from contextlib import ExitStack
        ctx = ExitStack()
        with ctx:
            pers = ctx.enter_context(tc.tile_pool(name="pers", bufs=1))
            dram = ctx.enter_context(tc.tile_pool(name="dram", bufs=1, space="DRAM"))

            # --- weights to SBUF ---
            w1b = pers.tile([128, 4, H1], bf)       # [d-part, dblk, f1]
            nc.sync.dma_start(w1b[:], w1_in.ap().rearrange("(db p) f -> p db f", p=128))
            w2b = pers.tile([128, 2, LAT], bf)      # [f-part, fblk, n]
            nc.sync.dma_start(w2b[:], w2_in.ap().rearrange("(fb p) n -> p fb n", p=128))
            wml = pers.tile([128, 2, LAT], bf)
            nc.sync.dma_start(wml[:, 0, :], wmu_in.ap())
            nc.sync.dma_start(wml[:, 1, :], wlv_in.ap())
            bias = pers.tile([128, 5], f32)  # [b1(2) | b2 | bmu | blv]
            nc.sync.dma_start(bias[:, 0:2], b1_in.ap().rearrange("(fb p) o -> p fb o", p=128))
            nc.sync.dma_start(bias[:, 2:3], b2_in.ap())
            nc.sync.dma_start(bias[:, 3:4], bmu_in.ap())
            nc.sync.dma_start(bias[:, 4:5], blv_in.ap())
            b1sb = bias[:, 0:2]
            b2sb = bias[:, 2:3]
            bmusb = bias[:, 3:4]
            blvsb = bias[:, 4:5]
            ident = pers.tile([128, 128], f32)
            make_identity(nc, ident[:])

            z1t = pers.tile([128, 2, OWN], bf)      # [f-part, fblk, own-i]
            zwall = pers.tile([128, NB, LAT], bf)   # [j-part, jb, n]
            ztb = pers.tile([128, OWN], bf)         # [f2-part, own-i]
            zwb = pers.tile([128, OWNB, LAT], bf)   # own zw, [j-part, jb, n]

            zw_bounce = dram.tile([OWN, LAT], bf)
            zw_all_d = dram.tile([N, LAT], bf, addr_space="Shared")
            mu_bounce = dram.tile([128, OWN], bf)
            mu_all_d = dram.tile([NC * 128, OWN], bf, addr_space="Shared")

            with tc.tile_pool(name="big", bufs=1) as big:
                stF = big.tile([128, NB, OWN], bf)  # [j-part, jb, own-i] 128K/part
                hb = big.tile([128, NB, H1], bf)    # [j-part, jb, f1]    32K/part

                # xt strips first (phase A input, 8 MB) on sync queue
                xts = []
                with tc.tile_pool(name="xt", bufs=4) as xpool:
                    for g in range(16):
                        xt = xpool.tile([128, 4, 512], bf, name=f"xt{g}",
                                        tag="xt")
                        nc.sync.dma_start(
                            xt[:],
                            xT_in.ap()[:, g * 512:(g + 1) * 512]
                            .rearrange("(db p) j -> p db j", p=128),
                        )
                        xts.append(xt)

                    # ST in 8 chunks of 8 jb (2 MB each) on vector queue
                    for q in range(8):
                        nc.scalar.dma_start(
                            stF[:, q * 8:(q + 1) * 8, :],
                            st_in.ap()[q * 1024:(q + 1) * 1024, :]
                            .rearrange("(jb p) i -> p jb i", p=128),
                        )

                    # --- phases A+B fused, jb-outer: h tiles feed conv1
                    #     accumulation chunk by chunk as ST arrives ---
                    with (
                        tc.tile_pool(name="pa", bufs=2, space="PSUM") as pa,
                        tc.tile_pool(name="pb", bufs=1, space="PSUM") as pb,
                    ):
                        pzs = [pb.tile([128, 512], f32, name=f"pz{i}")
                               for i in range(4)]
                        for q in range(9):
                            # A for chunk q (8 jb), conv1 for chunk q-1
                            if q < 8:
                                for jj in range(8):
                                    jb = q * 8 + jj
                                    xt = xts[jb // 4]
                                    ph = pa.tile([128, H1], f32, name="ph",
                                                 tag="ph")
                                    for db in range(4):
                                        nc.tensor.matmul(
                                            ph[:],
                                            xt[:, db,
                                               (jb % 4) * 128:(jb % 4 + 1) * 128],
                                            w1b[:, db, :],
                                            start=(db == 0),
                                            stop=(db == 3),
                                        )
                                    nc.vector.tensor_copy(hb[:, jb, :], ph[:])
                            if q > 0:
                                for jj in range(8):
                                    jb = (q - 1) * 8 + jj
                                    for fblk in range(2):
                                        for ch in range(2):
                                            nc.tensor.matmul(
                                                pzs[fblk * 2 + ch][:],
                                                hb[:, jb,
                                                   fblk * 128:(fblk + 1) * 128],
                                                stF[:, jb,
                                                    ch * 512:(ch + 1) * 512],
                                                start=(jb == 0),
                                                stop=(jb == NB - 1),
                                            )
                        for fblk in range(2):
                            for ch in range(2):
                                nc.scalar.activation(
                                    z1t[:, fblk, ch * 512:(ch + 1) * 512],
                                    pzs[fblk * 2 + ch][:],
                                    AF.Relu,
                                    bias=b1sb[:, fblk:fblk + 1],
                                )

                # --- phase C: zw_c = z1_c @ W2 ; AllGather ---
                with tc.tile_pool(name="pc", bufs=2, space="PSUM") as pc:
                    for jj in range(OWNB):
                        pw = pc.tile([128, LAT], f32)
                        for fblk in range(2):
                            nc.tensor.matmul(
                                pw[:],
                                z1t[:, fblk, jj * 128:(jj + 1) * 128],
                                w2b[:, fblk, :],
                                start=(fblk == 0),
                                stop=(fblk == 1),
                            )
                        nc.vector.tensor_copy(zwb[:, jj, :], pw[:])
                nc.gpsimd.dma_start(
                    zw_bounce[:].rearrange("(jb p) n -> p jb n", p=128), zwb[:]
                )
                nc.gpsimd.collective_compute(
                    "AllGather",
                    bass.mybir.AluOpType.bypass,
                    replica_groups=rg,
                    ins=[zw_bounce.opt()],
                    outs=[zw_all_d.opt()],
                )
                nc.gpsimd.dma_start(
                    zwall[:], zw_all_d[:].rearrange("(jb p) n -> p jb n", p=128)
                )

                # --- phase D: conv2 -> zT = sum_j zw[j]^T . ST[j] + b2 ---
                with tc.tile_pool(name="pd", bufs=2, space="PSUM") as pd:
                    for ch in range(2):
                        pz2 = pd.tile([128, 512], f32)
                        for jb in range(NB):
                            nc.tensor.matmul(
                                pz2[:],
                                zwall[:, jb, :],
                                stF[:, jb, ch * 512:(ch + 1) * 512],
                                start=(jb == 0),
                                stop=(jb == NB - 1),
                            )
                        nc.scalar.activation(
                            ztb[:, ch * 512:(ch + 1) * 512],
                            pz2[:],
                            AF.Identity,
                            bias=b2sb,
                        )

            # big pool (ST, hb) released here
            with tc.tile_pool(name="post", bufs=1) as post:
                muT = post.tile([128, OWN], f32)
                lvT = post.tile([128, OWN], f32)
                muTb = post.tile([128, OWN], bf)
                musb = post.tile([128, OWNB, LAT], f32)
                lvsb = post.tile([128, OWNB, LAT], f32)
                muall = post.tile([128, NC, OWN], bf)

                # --- phase E: muT/lvT + transposes ---
                with tc.tile_pool(name="pe", bufs=4, space="PSUM") as pe:
                    for ch in range(2):
                        pm = pe.tile([128, 512], f32)
                        nc.tensor.matmul(
                            pm[:], wml[:, 0, :], ztb[:, ch * 512:(ch + 1) * 512],
                            start=True, stop=True,
                        )
                        nc.scalar.activation(
                            muT[:, ch * 512:(ch + 1) * 512], pm[:],
                            AF.Identity, bias=bmusb,
                        )
                        pl = pe.tile([128, 512], f32)
                        nc.tensor.matmul(
                            pl[:], wml[:, 1, :], ztb[:, ch * 512:(ch + 1) * 512],
                            start=True, stop=True,
                        )
                        nc.scalar.activation(
                            lvT[:, ch * 512:(ch + 1) * 512], pl[:],
                            AF.Identity, bias=blvsb,
                        )
                    nc.vector.tensor_copy(muTb[:], muT[:])

                with tc.tile_pool(name="pt", bufs=4, space="PSUM") as pt:
                    for jj in range(OWNB):
                        ptm = pt.tile([128, 128], f32)
                        nc.tensor.transpose(
                            ptm[:], muT[:, jj * 128:(jj + 1) * 128], ident[:]
                        )
                        nc.vector.tensor_copy(musb[:, jj, :], ptm[:])
                        ptl = pt.tile([128, 128], f32)
                        nc.tensor.transpose(
                            ptl[:], lvT[:, jj * 128:(jj + 1) * 128], ident[:]
                        )
                        nc.vector.tensor_copy(lvsb[:, jj, :], ptl[:])
                nc.sync.dma_start(
                    mu_out.ap().rearrange("(jb p) n -> p jb n", p=128), musb[:]
                )
                nc.sync.dma_start(
                    lv_out.ap().rearrange("(jb p) n -> p jb n", p=128), lvsb[:]
                )

                nc.gpsimd.dma_start(mu_bounce[:], muTb[:])
                nc.gpsimd.collective_compute(
                    "AllGather",
                    bass.mybir.AluOpType.bypass,
                    replica_groups=rg,
                    ins=[mu_bounce.opt()],
                    outs=[mu_all_d.opt()],
                )
                for r8 in range(NC):
                    nc.gpsimd.dma_start(
                        muall[:, r8, :],
                        mu_all_d[r8 * 128:(r8 + 1) * 128, :],
                    )

                # --- phase F: adj_c = sigmoid(muT_c^T . muT_all) ---
                with (
                    tc.tile_pool(name="adjp", bufs=3) as adjp,
                    tc.tile_pool(name="pf", bufs=4, space="PSUM") as pf,
                ):
                    for iblk in range(OWNB):
                        for cg in range(4):  # column groups of 2048
                            adjch = adjp.tile([128, 2048], f32)
                            for s in range(4):
                                col = cg * 2048 + s * 512
                                r8, ch2 = col // 1024, (col % 1024) // 512
                                pa2 = pf.tile([128, 512], f32)
                                nc.tensor.matmul(
                                    pa2[:],
                                    muTb[:, iblk * 128:(iblk + 1) * 128],
                                    muall[:, r8, ch2 * 512:(ch2 + 1) * 512],
                                    start=True, stop=True,
                                )
                                nc.scalar.activation(
                                    adjch[:, s * 512:(s + 1) * 512], pa2[:],
                                    AF.Sigmoid,
                                )
                            nc.sync.dma_start(
                                adj_out.ap()[iblk * 128:(iblk + 1) * 128,
                                             cg * 2048:(cg + 1) * 2048],
                                adjch[:],
                            )

    nc.compile()
    return nc


def _prep_host(x, edge_index, W1, b1, W2, b2, Wmu, bmu, Wlv, blv):
    x = np.asarray(x)
    e = np.asarray(edge_index)
    sl = np.arange(N, dtype=e.dtype)
    r = np.concatenate([e[0], sl, sl])
    c = np.concatenate([e[1], sl, sl])
    deg = np.bincount(c, minlength=N).astype(np.float32)
    dis = np.where(deg > 0, deg ** -0.5, 0.0).astype(np.float32)
    vals = dis[r] * dis[c]
    ST = np.zeros((N, N), dtype=np.float32)
    np.add.at(ST, (r, c), vals)
    STb = ST.astype(BF16)
    xT = np.ascontiguousarray(np.asarray(x, np.float32).T).astype(BF16)

    common = {
        "xT": xT,
        "W1": np.asarray(W1, np.float32).astype(BF16),
        "W2": np.asarray(W2, np.float32).astype(BF16),
        "Wmu": np.asarray(Wmu, np.float32).astype(BF16),
        "Wlv": np.asarray(Wlv, np.float32).astype(BF16),
        "b1": np.asarray(b1, np.float32).reshape(H1, 1),
        "b2": np.asarray(b2, np.float32).reshape(LAT, 1),
        "bmu": np.asarray(bmu, np.float32).reshape(LAT, 1),
        "blv": np.asarray(blv, np.float32).reshape(LAT, 1),
    }
    in_maps = []
    for cidx in range(NC):
        m = dict(common)
        m["STc"] = np.ascontiguousarray(STb[:, cidx * OWN:(cidx + 1) * OWN])
        in_maps.append(m)
    return in_maps


def kernel(x, edge_index, W1, b1, W2, b2, Wmu, bmu, Wlv, blv):
    global LAST_RESULT
    _install_ntff_shim()
    from concourse import bass_utils

    if "nc" not in _CACHE:
        _CACHE["nc"] = _build_module()
    nc = _CACHE["nc"]

    in_maps = _prep_host(x, edge_index, W1, b1, W2, b2, Wmu, bmu, Wlv, blv)
    res = bass_utils.run_bass_kernel_spmd(nc, in_maps, core_ids=list(range(NC)))
    LAST_RESULT = res

    adj = np.concatenate([res.results[c]["adj"] for c in range(NC)], axis=0)
    mu = np.concatenate([res.results[c]["mu"] for c in range(NC)], axis=0)
    lv = np.concatenate([res.results[c]["lv"] for c in range(NC)], axis=0)
    return adj, mu, lv


# revision 11
# speedup vs baseline: 1.2005x; 1.0871x over previous
"""GraphVAE (2x GCNConv + inner-product decoder) on 8 TRN2 NeuronCores.

Strategy:
  - Host: build dense normalized adjacency ST[j,i] = sum_{e:(j->i)} dis[j]*dis[i]
    (includes the doubled self-loops), cast to bf16, shard columns (= output
    nodes) across the 8 cores. Transpose x to feat-major bf16.
  - Device (SPMD, per core c owning nodes I_c = [1024c, 1024(c+1))):
      h    = x @ W1                      (replicated, node-major bf16)
      z1T  = relu(sum_j h[j] ST_c[j] + b1)   (feat-major, own cols)
      zw_c = z1_c @ W2                   -> AllGather -> zw (node-major)
      zT   = sum_j zw[j] ST_c[j] + b2    (feat-major)
      muT  = Wmu.T zT + bmu ; lvT = Wlv.T zT + blv
      mu_c / lv_c via PE transpose (fp32 outputs)
      muT bf16 -> AllGather -> muT_all
      adj_c = sigmoid(muT_c.T @ muT_all)  (1024 x 8192, fp32 out)
All matmuls bf16 with fp32 PSUM accumulation.
"""

import os
import sys
import types

import ml_dtypes
import numpy as np

N = 8192
D_IN = 512
H1 = 256
LAT = 128
NC = 8
NB = N // 128          # 64 node blocks of 128
OWN = N // NC          # 1024 nodes per core
OWNB = OWN // 128      # 8 own node blocks

BF16 = ml_dtypes.bfloat16

_CACHE = {}
LAST_RESULT = None


def _install_ntff_shim():
    # Allows BASS_TRACE=1 profiling under axon (optional; harmless otherwise).
    try:
        import antenv.axon_hooks  # noqa: F401
        return
    except ImportError:
        pass
    try:
        from trn_agent_boot.trn_boot import _ntff_profile_via_ctypes
        hook = _ntff_profile_via_ctypes("/opt/axon/libaxon_pjrt.so")
        mod = types.ModuleType("antenv.axon_hooks")
        mod.get_axon_ntff_profile_hook = lambda: hook
        sys.modules["antenv.axon_hooks"] = mod
    except Exception:
        pass


def _build_module():
    import concourse.bass as bass
    import concourse.tile as tile
    from concourse import bacc, mybir
    from concourse.masks import make_identity

    f32 = mybir.dt.float32
    bf = mybir.dt.bfloat16
    AF = mybir.ActivationFunctionType

    nc = bacc.Bacc("TRN2", target_bir_lowering=False, debug=False, num_devices=NC)

    xT_in = nc.dram_tensor("xT", [D_IN, N], bf, kind="ExternalInput")
    st_in = nc.dram_tensor("STc", [N, OWN], bf, kind="ExternalInput")
    w1_in = nc.dram_tensor("W1", [D_IN, H1], bf, kind="ExternalInput")
    w2_in = nc.dram_tensor("W2", [H1, LAT], bf, kind="ExternalInput")
    wmu_in = nc.dram_tensor("Wmu", [LAT, LAT], bf, kind="ExternalInput")
    wlv_in = nc.dram_tensor("Wlv", [LAT, LAT], bf, kind="ExternalInput")
    b1_in = nc.dram_tensor("b1", [H1, 1], f32, kind="ExternalInput")
    b2_in = nc.dram_tensor("b2", [LAT, 1], f32, kind="ExternalInput")
    bmu_in = nc.dram_tensor("bmu", [LAT, 1], f32, kind="ExternalInput")
    blv_in = nc.dram_tensor("blv", [LAT, 1], f32, kind="ExternalInput")

    adj_out = nc.dram_tensor("adj", [OWN, N], f32, kind="ExternalOutput")
    mu_out = nc.dram_tensor("mu", [OWN, LAT], f32, kind="ExternalOutput")
    lv_out = nc.dram_tensor("lv", [OWN, LAT], f32, kind="ExternalOutput")

    rg = [list(range(NC))]

    with tile.TileContext(nc) as tc:
        from contextlib import ExitStack
        ctx = ExitStack()
        with ctx:
            pers = ctx.enter_context(tc.tile_pool(name="pers", bufs=1))
            dram = ctx.enter_context(tc.tile_pool(name="dram", bufs=1, space="DRAM"))

            # --- weights to SBUF ---
            w1b = pers.tile([128, 4, H1], bf)       # [d-part, dblk, f1]
            nc.sync.dma_start(w1b[:], w1_in.ap().rearrange("(db p) f -> p db f", p=128))
            w2b = pers.tile([128, 2, LAT], bf)      # [f-part, fblk, n]
            nc.sync.dma_start(w2b[:], w2_in.ap().rearrange("(fb p) n -> p fb n", p=128))
            wml = pers.tile([128, 2, LAT], bf)
            nc.sync.dma_start(wml[:, 0, :], wmu_in.ap())
            nc.sync.dma_start(wml[:, 1, :], wlv_in.ap())
            bias = pers.tile([128, 5], f32)  # [b1(2) | b2 | bmu | blv]
            nc.sync.dma_start(bias[:, 0:2], b1_in.ap().rearrange("(fb p) o -> p fb o", p=128))
            nc.sync.dma_start(bias[:, 2:3], b2_in.ap())
            nc.sync.dma_start(bias[:, 3:4], bmu_in.ap())
            nc.sync.dma_start(bias[:, 4:5], blv_in.ap())
            b1sb = bias[:, 0:2]
            b2sb = bias[:, 2:3]
            bmusb = bias[:, 3:4]
            blvsb = bias[:, 4:5]
            ident = pers.tile([128, 128], f32)
            make_identity(nc, ident[:])

            z1t = pers.tile([128, 2, OWN], bf)      # [f-part, fblk, own-i]
            zwall = pers.tile([128, NB, LAT], bf)   # [j-part, jb, n]
            ztb = pers.tile([128, OWN], bf)         # [f2-part, own-i]
            zwb = pers.tile([128, OWNB, LAT], bf)   # own zw, [j-part, jb, n]

            zw_bounce = dram.tile([OWN, LAT], bf)
            zw_all_d = dram.tile([N, LAT], bf, addr_space="Shared")
            mu_bounce = dram.tile([128, OWN], bf)
            mu_all_d = dram.tile([NC * 128, OWN], bf, addr_space="Shared")

            with tc.tile_pool(name="big", bufs=1) as big:
                stF = big.tile([128, NB, OWN], bf)  # [j-part, jb, own-i] 128K/part
                hb = big.tile([128, NB, H1], bf)    # [j-part, jb, f1]    32K/part

                # xt strips first (phase A input, 8 MB) on sync queue
                xts = []
                with tc.tile_pool(name="xt", bufs=4) as xpool:
                    for g in range(16):
                        xt = xpool.tile([128, 4, 512], bf, name=f"xt{g}",
                                        tag="xt")
                        nc.sync.dma_start(
                            xt[:],
                            xT_in.ap()[:, g * 512:(g + 1) * 512]
                            .rearrange("(db p) j -> p db j", p=128),
                        )
                        xts.append(xt)

                    # ST in 8 chunks of 8 jb (2 MB each) on vector queue
                    for q in range(8):
                        eng = nc.scalar if q % 2 == 0 else nc.sync
                        eng.dma_start(
                            stF[:, q * 8:(q + 1) * 8, :],
                            st_in.ap()[q * 1024:(q + 1) * 1024, :]
                            .rearrange("(jb p) i -> p jb i", p=128),
                        )

                    # --- phases A+B fused, jb-outer: h tiles feed conv1
                    #     accumulation chunk by chunk as ST arrives ---
                    with (
                        tc.tile_pool(name="pa", bufs=2, space="PSUM") as pa,
                        tc.tile_pool(name="pb", bufs=1, space="PSUM") as pb,
                    ):
                        pzs = [pb.tile([128, 512], f32, name=f"pz{i}")
                               for i in range(4)]
                        for q in range(9):
                            # A for chunk q (8 jb), conv1 for chunk q-1
                            if q < 8:
                                for jj in range(8):
                                    jb = q * 8 + jj
                                    xt = xts[jb // 4]
                                    ph = pa.tile([128, H1], f32, name="ph",
                                                 tag="ph")
                                    for db in range(4):
                                        nc.tensor.matmul(
                                            ph[:],
                                            xt[:, db,
                                               (jb % 4) * 128:(jb % 4 + 1) * 128],
                                            w1b[:, db, :],
                                            start=(db == 0),
                                            stop=(db == 3),
                                        )
                                    nc.vector.tensor_copy(hb[:, jb, :], ph[:])
                            if q > 0:
                                for jj in range(8):
                                    jb = (q - 1) * 8 + jj
                                    for fblk in range(2):
                                        for ch in range(2):
                                            nc.tensor.matmul(
                                                pzs[fblk * 2 + ch][:],
                                                hb[:, jb,
                                                   fblk * 128:(fblk + 1) * 128],
                                                stF[:, jb,
                                                    ch * 512:(ch + 1) * 512],
                                                start=(jb == 0),
                                                stop=(jb == NB - 1),
                                            )
                        for fblk in range(2):
                            for ch in range(2):
                                nc.scalar.activation(
                                    z1t[:, fblk, ch * 512:(ch + 1) * 512],
                                    pzs[fblk * 2 + ch][:],
                                    AF.Relu,
                                    bias=b1sb[:, fblk:fblk + 1],
                                )

                # --- phase C: zw_c = z1_c @ W2 ; AllGather ---
                with tc.tile_pool(name="pc", bufs=2, space="PSUM") as pc:
                    for jj in range(OWNB):
                        pw = pc.tile([128, LAT], f32)
                        for fblk in range(2):
                            nc.tensor.matmul(
                                pw[:],
                                z1t[:, fblk, jj * 128:(jj + 1) * 128],
                                w2b[:, fblk, :],
                                start=(fblk == 0),
                                stop=(fblk == 1),
                            )
                        nc.vector.tensor_copy(zwb[:, jj, :], pw[:])
                nc.sync.dma_start(
                    zw_bounce[:].rearrange("(jb p) n -> p jb n", p=128), zwb[:]
                )
                nc.gpsimd.collective_compute(
                    "AllGather",
                    bass.mybir.AluOpType.bypass,
                    replica_groups=rg,
                    ins=[zw_bounce.opt()],
                    outs=[zw_all_d.opt()],
                )
                nc.scalar.dma_start(
                    zwall[:], zw_all_d[:].rearrange("(jb p) n -> p jb n", p=128)
                )

                # --- phase D: conv2 -> zT = sum_j zw[j]^T . ST[j] + b2 ---
                with tc.tile_pool(name="pd", bufs=2, space="PSUM") as pd:
                    for ch in range(2):
                        pz2 = pd.tile([128, 512], f32)
                        for jb in range(NB):
                            nc.tensor.matmul(
                                pz2[:],
                                zwall[:, jb, :],
                                stF[:, jb, ch * 512:(ch + 1) * 512],
                                start=(jb == 0),
                                stop=(jb == NB - 1),
                            )
                        nc.scalar.activation(
                            ztb[:, ch * 512:(ch + 1) * 512],
                            pz2[:],
                            AF.Identity,
                            bias=b2sb,
                        )

            # big pool (ST, hb) released here
            with tc.tile_pool(name="post", bufs=1) as post:
                muT = post.tile([128, OWN], f32)
                lvT = post.tile([128, OWN], f32)
                muTb = post.tile([128, OWN], bf)
                musb = post.tile([128, OWNB, LAT], f32)
                lvsb = post.tile([128, OWNB, LAT], f32)
                muall = post.tile([128, NC, OWN], bf)

                # --- phase E: muT/lvT + transposes ---
                with tc.tile_pool(name="pe", bufs=4, space="PSUM") as pe:
                    for ch in range(2):
                        pm = pe.tile([128, 512], f32)
                        nc.tensor.matmul(
                            pm[:], wml[:, 0, :], ztb[:, ch * 512:(ch + 1) * 512],
                            start=True, stop=True,
                        )
                        nc.scalar.activation(
                            muT[:, ch * 512:(ch + 1) * 512], pm[:],
                            AF.Identity, bias=bmusb,
                        )
                        pl = pe.tile([128, 512], f32)
                        nc.tensor.matmul(
                            pl[:], wml[:, 1, :], ztb[:, ch * 512:(ch + 1) * 512],
                            start=True, stop=True,
                        )
                        nc.scalar.activation(
                            lvT[:, ch * 512:(ch + 1) * 512], pl[:],
                            AF.Identity, bias=blvsb,
                        )
                    nc.vector.tensor_copy(muTb[:], muT[:])

                with tc.tile_pool(name="pt", bufs=4, space="PSUM") as pt:
                    for jj in range(OWNB):
                        ptm = pt.tile([128, 128], f32)
                        nc.tensor.transpose(
                            ptm[:], muT[:, jj * 128:(jj + 1) * 128], ident[:]
                        )
                        nc.vector.tensor_copy(musb[:, jj, :], ptm[:])
                        ptl = pt.tile([128, 128], f32)
                        nc.tensor.transpose(
                            ptl[:], lvT[:, jj * 128:(jj + 1) * 128], ident[:]
                        )
                        nc.vector.tensor_copy(lvsb[:, jj, :], ptl[:])
                nc.sync.dma_start(
                    mu_out.ap().rearrange("(jb p) n -> p jb n", p=128), musb[:]
                )
                nc.sync.dma_start(
                    lv_out.ap().rearrange("(jb p) n -> p jb n", p=128), lvsb[:]
                )

                nc.sync.dma_start(mu_bounce[:], muTb[:])
                nc.gpsimd.collective_compute(
                    "AllGather",
                    bass.mybir.AluOpType.bypass,
                    replica_groups=rg,
                    ins=[mu_bounce.opt()],
                    outs=[mu_all_d.opt()],
                )
                for r8 in range(NC):
                    eng = nc.scalar if r8 % 2 == 0 else nc.sync
                    eng.dma_start(
                        muall[:, r8, :],
                        mu_all_d[r8 * 128:(r8 + 1) * 128, :],
                    )

                # --- phase F: adj_c = sigmoid(muT_c^T . muT_all) ---
                with (
                    tc.tile_pool(name="adjp", bufs=3) as adjp,
                    tc.tile_pool(name="pf", bufs=4, space="PSUM") as pf,
                ):
                    for iblk in range(OWNB):
                        for cg in range(4):  # column groups of 2048
                            adjch = adjp.tile([128, 2048], f32)
                            for s in range(4):
                                col = cg * 2048 + s * 512
                                r8, ch2 = col // 1024, (col % 1024) // 512
                                pa2 = pf.tile([128, 512], f32)
                                nc.tensor.matmul(
                                    pa2[:],
                                    muTb[:, iblk * 128:(iblk + 1) * 128],
                                    muall[:, r8, ch2 * 512:(ch2 + 1) * 512],
                                    start=True, stop=True,
                                )
                                nc.scalar.activation(
                                    adjch[:, s * 512:(s + 1) * 512], pa2[:],
                                    AF.Sigmoid,
                                )
                            eng = nc.sync if (iblk * 4 + cg) % 2 == 0 else nc.scalar
                            eng.dma_start(
                                adj_out.ap()[iblk * 128:(iblk + 1) * 128,
                                             cg * 2048:(cg + 1) * 2048],
                                adjch[:],
                            )

    nc.compile()
    return nc


def _prep_host(x, edge_index, W1, b1, W2, b2, Wmu, bmu, Wlv, blv):
    x = np.asarray(x)
    e = np.asarray(edge_index)
    sl = np.arange(N, dtype=e.dtype)
    r = np.concatenate([e[0], sl, sl])
    c = np.concatenate([e[1], sl, sl])
    deg = np.bincount(c, minlength=N).astype(np.float32)
    dis = np.where(deg > 0, deg ** -0.5, 0.0).astype(np.float32)
    vals = dis[r] * dis[c]
    ST = np.zeros((N, N), dtype=np.float32)
    np.add.at(ST, (r, c), vals)
    STb = ST.astype(BF16)
    xT = np.ascontiguousarray(np.asarray(x, np.float32).T).astype(BF16)

    common = {
        "xT": xT,
        "W1": np.asarray(W1, np.float32).astype(BF16),
        "W2": np.asarray(W2, np.float32).astype(BF16),
        "Wmu": np.asarray(Wmu, np.float32).astype(BF16),
        "Wlv": np.asarray(Wlv, np.float32).astype(BF16),
        "b1": np.asarray(b1, np.float32).reshape(H1, 1),
        "b2": np.asarray(b2, np.float32).reshape(LAT, 1),
        "bmu": np.asarray(bmu, np.float32).reshape(LAT, 1),
        "blv": np.asarray(blv, np.float32).reshape(LAT, 1),
    }
    in_maps = []
    for cidx in range(NC):
        m = dict(common)
        m["STc"] = np.ascontiguousarray(STb[:, cidx * OWN:(cidx + 1) * OWN])
        in_maps.append(m)
    return in_maps


def kernel(x, edge_index, W1, b1, W2, b2, Wmu, bmu, Wlv, blv):
    global LAST_RESULT
    _install_ntff_shim()
    from concourse import bass_utils

    if "nc" not in _CACHE:
        _CACHE["nc"] = _build_module()
    nc = _CACHE["nc"]

    in_maps = _prep_host(x, edge_index, W1, b1, W2, b2, Wmu, bmu, Wlv, blv)
    res = bass_utils.run_bass_kernel_spmd(nc, in_maps, core_ids=list(range(NC)))
    LAST_RESULT = res

    adj = np.concatenate([res.results[c]["adj"] for c in range(NC)], axis=0)
    mu = np.concatenate([res.results[c]["mu"] for c in range(NC)], axis=0)
    lv = np.concatenate([res.results[c]["lv"] for c in range(NC)], axis=0)
    return adj, mu, lv
